# revision 1
# baseline (speedup 1.0000x reference)
"""AdaptiveBlock Trainium2 kernel, 8-core data-parallel.

Reference (per batch): y = mean(x, HW); y' = gelu(gelu(y@w1.T)@w2.T);
attn = sigmoid((y'@wA.T).reshape(H,R) @ (y'@wB.T).reshape(R,W));
out = attn broadcast over C.

Sharding: B=32 over 8 cores (4 batches/core), weights replicated.
Memory-bound: 13.9 MB read + 12.8 MB written per core; per-core DMA
ceiling ~427 GB/s (16 engines x 26.7 GB/s for any packet >= 6 KB).

Schedule (measured-driven; see inline comments):
- ALL big HBM traffic rides the single Sync HWDGE queue in program
  order (weights, x chunks, then outputs): in-queue FIFO = automatic
  read priority, writes backfill the instant reads drain.  SWDGE
  (gpsimd-queue) reads sagged ~25% from DRAM descriptor fetches.
- Each batch computes SOLO: its MLP starts ~4 us after its own last
  chunk's completion semaphore (sem delivery lags data ~4 us, fixed),
  so the long flat->broadcast handoff overlaps the read stream.
- Spatial sums: cc0 chunk on DVE reduce, cc1 on ACT accum; the last
  chunk (b3 cc1) splits across both engines to shorten the tail.
- The bilinear A@B is 8 accumulating rank-1 (K=1) matmuls reading
  slices of the batch's A|B row in place (64-aligned host-side weight
  permutation) -- no cross-partition operand extraction.
- sigmoid = Tanh activation + Copy-with-scale/bias affine, both ACT:
  Gelu+Tanh tables coexist in table RAM; Gelu+Sigmoid thrash (1.28 us
  reload per switch on the critical engine).
- The (56,56)->(1,3136) flatten DMA rides the scalar HWDGE queue in
  two 28-packet pieces; SWDGE semaphores cost ~4 us extra.
- Broadcast to 128 partitions: gpsimd partition_broadcast in column
  halves (cost ~1.6 ns per free element; 128-partition output is
  mandatory -- a 64-partition source halves the SBUF read fabric and
  the write stream with it).  PE ones-matmul broadcast is not used:
  it saturates PE and the readiness-order scheduler starves deadline
  work.
- Output DMAs are column-halved so the first half flies while the
  second still broadcasts; out triggers are emitted strictly after
  all read triggers (sync engine would stall head-of-line otherwise).

Numerics: weights pre-transposed/pre-cast to bf16 host-side, PSUM f32,
activations ~1e-2, tanh values ~1e-4 (bf16/f32-precise near 0): final
rel err ~7e-7 vs the f32 reference.
"""

import numpy as np
import ml_dtypes

import concourse.bass as bass
import concourse.tile as tile
from concourse import bacc, mybir
from concourse.bass_utils import run_bass_kernel_spmd

F32 = mybir.dt.float32
BF16 = mybir.dt.bfloat16

B, C, H, W = 32, 256, 56, 56
HW = H * W                      # 3136
HIDDEN = 512
RANK = 8
HR = H * RANK                   # 448
NCORES = 8
BLOC = B // NCORES              # 4 batches per core
P = 128
NCC = C // P                    # 2 channel chunks
NHH = HIDDEN // P               # 4 hidden chunks
BCHUNK = 512                    # matmul moving free-dim max (PSUM bank)
# batch -> (group, slot): group A = (b0, b1) computed mid-stream; b2 and
# b3 run solo so neither waits on later data -- b3's chain is the only
# one after the read stream ends, and its output needs to be ready a
# full ~12 us before the write stream would drain dry
GROUPS = [[0], [1], [2], [3]]
GMAP = {0: (0, 0), 1: (1, 0), 2: (2, 0), 3: (3, 0)}
NGRP = len(GROUPS)


def build_bass(sim_compat: bool = False) -> bacc.Bacc:
    """sim_compat=True swaps exact Gelu (not implemented in CoreSim) for a
    0.5*x stand-in; with |gelu-input| ~ 0.02 this perturbs the final sigmoid
    output by ~1e-5 relative, so the sim still validates all layout/dataflow.
    Hardware builds always use the exact erf-based Gelu."""
    gelu_f = (
        mybir.ActivationFunctionType.Copy
        if sim_compat
        else mybir.ActivationFunctionType.Gelu
    )
    gelu_s = 0.5 if sim_compat else 1.0
    nc = bacc.Bacc(num_devices=NCORES)

    HRP = 2 * C                                        # 512: r*64+i padded cols
    WPACK = NCC * HIDDEN + NHH * C + 2 * NCC * HRP     # 4096 bf16 columns
    x_d = nc.dram_tensor("x", [BLOC, C, HW], F32, kind="ExternalInput")
    wpk_d = nc.dram_tensor("wpk", [P, WPACK], BF16, kind="ExternalInput")
    out_d = nc.dram_tensor("out", [BLOC, C, HW], F32, kind="ExternalOutput")

    x_v = x_d.ap().rearrange("b (cc p) hw -> b cc p hw", p=P)
    out_v = out_d.ap().rearrange("b (cc p) hw -> b cc p hw", p=P)
    OF_W1 = 0
    OF_W2 = NCC * HIDDEN
    OF_WA = NCC * HIDDEN + NHH * C
    OF_WB = NCC * HIDDEN + NHH * C + NCC * HRP

    with tile.TileContext(nc) as tc:
        with (
            tc.tile_pool(name="xin", bufs=8) as xpool,
            tc.tile_pool(name="persist", bufs=1) as ppool,
            tc.tile_pool(name="small", bufs=2) as spool,
            tc.tile_pool(name="bc", bufs=4) as bcpool,
            tc.tile_pool(name="ps_small", bufs=2, space="PSUM") as ps_small,
            tc.tile_pool(name="ps_ab", bufs=2, space="PSUM") as ps_ab,
        ):
            wpk = ppool.tile([P, WPACK], BF16, tag="wpk", name="wpk")
            w1t = [wpk[:, OF_W1 + cc * HIDDEN : OF_W1 + (cc + 1) * HIDDEN]
                   for cc in range(NCC)]
            w2t = [wpk[:, OF_W2 + hh * C : OF_W2 + (hh + 1) * C]
                   for hh in range(NHH)]
            wat = [wpk[:, OF_WA + cc * HRP : OF_WA + (cc + 1) * HRP]
                   for cc in range(NCC)]
            wbt = [wpk[:, OF_WB + cc * HRP : OF_WB + (cc + 1) * HRP]
                   for cc in range(NCC)]
            ones = ppool.tile([1, P], BF16, tag="ones", name="ones")
            nc.vector.memset(ones[:], 1.0)

            # prefetch both activation tables while ACT is idle
            warm = ppool.tile([1, 2], F32, tag="warm", name="warm")
            nc.vector.memset(warm[:], 0.0)
            nc.scalar.activation(warm[:], warm[:],
                                 mybir.ActivationFunctionType.Tanh)
            nc.scalar.activation(warm[:], warm[:], gelu_f)

            ysum = [[ppool.tile([P, len(GROUPS[g])], F32,
                                tag=f"ysum{g}{cc}", name=f"ysum{g}{cc}")
                     for cc in range(NCC)] for g in range(NGRP)]
            def half_reduce(dst, view, eng):
                if eng == "vector":
                    nc.vector.reduce_sum(dst, view, axis=mybir.AxisListType.X)
                else:
                    nc.scalar.activation(
                        view, view, mybir.ActivationFunctionType.Copy,
                        accum_out=dst,
                    )

            def load_chunk(b, cc, split=None):
                """split=(eng_h1, eng_h2): two half DMAs + half reduces.
                A full chunk's completion semaphore lags its nominal end by
                ~4us (per-transfer sems trail the whole 128-packet spread);
                half-chunk sems fire ~2us apart, so a split chain gates
                ~3-4us earlier.  Used where the reduce gates a compute
                chain: b1 (group A's MLP) and b3 (the post-stream tail)."""
                g, j = GMAP[b]
                xt = xpool.tile([P, HW], F32, tag="xt", name="xt")
                if split is None:
                    nc.sync.dma_start(xt[:], x_v[b, cc])
                    half_reduce(ysum[g][cc][:, j : j + 1], xt[:],
                                "vector" if cc == 0 else "scalar")
                else:
                    HH = HW // 2
                    hp = ppool.tile([P, 2], F32, tag=f"hp{b}{cc}",
                                    name=f"hp{b}{cc}")
                    nc.sync.dma_start(xt[:, 0:HH], x_v[b, cc][:, 0:HH])
                    half_reduce(hp[:, 0:1], xt[:, 0:HH], split[0])
                    nc.sync.dma_start(xt[:, HH:HW], x_v[b, cc][:, HH:HW])
                    half_reduce(hp[:, 1:2], xt[:, HH:HW], split[1])
                    nc.vector.tensor_add(
                        ysum[g][cc][:, j : j + 1], hp[:, 0:1], hp[:, 1:2],
                    )

            def make_ysb(g, eng):
                gb = len(GROUPS[g])
                ysb = [ppool.tile([P, gb], BF16, tag=f"ysb{g}{cc}",
                                  name=f"ysb{g}{cc}") for cc in range(NCC)]
                for cc in range(NCC):
                    if eng == "scalar":
                        nc.scalar.copy(ysb[cc][:], ysum[g][cc][:])
                    else:
                        nc.vector.tensor_copy(ysb[cc][:], ysum[g][cc][:])
                return ysb

            def mlp_group(g, ysb):
                """MLP + per-batch A|B row for one (solo) batch group."""
                gb = len(GROUPS[g])
                hT = [ppool.tile([P, gb], BF16, tag=f"hT{g}{hh}",
                                 name=f"hT{g}{hh}") for hh in range(NHH)]
                for hh in range(NHH):
                    ph = ps_small.tile([P, gb], F32, tag="ps", name="ps")
                    for cc in range(NCC):
                        nc.tensor.matmul(
                            ph[:], w1t[cc][:, hh * P : (hh + 1) * P], ysb[cc][:],
                            start=(cc == 0), stop=(cc == NCC - 1),
                        )
                    nc.scalar.activation(hT[hh][:], ph[:], gelu_f,
                                         scale=gelu_s / HW)
                ypT = [ppool.tile([P, gb], BF16, tag=f"ypT{g}{cc}",
                                  name=f"ypT{g}{cc}") for cc in range(NCC)]
                for cc in range(NCC):
                    pyp = ps_small.tile([P, gb], F32, tag="ps", name="ps")
                    for hh in range(NHH):
                        nc.tensor.matmul(
                            pyp[:], w2t[hh][:, cc * P : (cc + 1) * P], hT[hh][:],
                            start=(hh == 0), stop=(hh == NHH - 1),
                        )
                    nc.scalar.activation(ypT[cc][:], pyp[:], gelu_f,
                                         scale=gelu_s)
                # A|B projection per batch (M=1) so each batch's row sits
                # at partition 0 (PE operand base-partition rule) and the
                # bilinear reads slices of this row directly
                abjs = []
                for j in range(gb):
                    pab = ps_ab.tile([1, 2 * BCHUNK], F32, tag="pab",
                                     name="pab")
                    for half, wt in ((0, wat), (1, wbt)):
                        for cc in range(NCC):
                            nc.tensor.matmul(
                                pab[:, half * BCHUNK : (half + 1) * BCHUNK],
                                ypT[cc][:, j : j + 1], wt[cc][:],
                                start=(cc == 0), stop=(cc == NCC - 1),
                            )
                    abj = ppool.tile([1, 2 * BCHUNK], BF16, tag=f"ab{g}{j}",
                                     name=f"ab{g}{j}")
                    # single-partition copies are free-dim serial: split
                    nc.scalar.copy(abj[:, 0:BCHUNK], pab[:, 0:BCHUNK])
                    nc.vector.tensor_copy(abj[:, BCHUNK:], pab[:, BCHUNK:])
                    abjs.append(abj)
                return abjs

            def bilinear_flat(abj):
                """tanh((A @ B)/2) flattened to (1, HW) bf16.  The tanh
                values are ~1e-4 (bf16-precise near 0); the sigmoid's
                0.5x+0.5 rides the broadcast PSUM->SBUF copies in f32.

                A @ B is 8 accumulating rank-1 (K=1) matmuls whose operands
                are slices of the batch's ab row itself -- the 64-aligned
                host-side weight permutation puts A[:, r] at cols r*64:+56
                of the first half and B[r, :] at the same cols of the
                second half, so no cross-partition extraction DMA."""
                pm = ps_small.tile([H, W], F32, tag="ps", name="ps")
                for r in range(RANK):
                    nc.tensor.matmul(
                        pm[:],
                        abj[0:1, r * 64 : r * 64 + H],
                        abj[0:1, BCHUNK + r * 64 : BCHUNK + r * 64 + W],
                        start=(r == 0), stop=(r == RANK - 1),
                    )
                msbt = spool.tile([H, W], F32, tag="msbt", name="msbt")
                nc.scalar.activation(msbt[:], pm[:],
                                     mybir.ActivationFunctionType.Tanh,
                                     scale=0.5)
                msb = spool.tile([H, W], F32, tag="msb", name="msb")
                # finishing sigmoid affine on ACT via Copy (no table)
                nc.scalar.activation(msb[:], msbt[:],
                                     mybir.ActivationFunctionType.Copy,
                                     bias=0.5, scale=0.5)
                flat = spool.tile([1, HW], F32, tag="flat", name="flat")
                # the 56-row flatten dribbles behind the big stream on any
                # queue -- split across BOTH small queues (28 packets each)
                flat_v = flat[0:1, :].rearrange("o (i j) -> o i j", i=H)
                nc.scalar.dma_start(flat_v[:, 0 : H // 2, :],
                                    msb[0 : H // 2, :])
                nc.scalar.dma_start(flat_v[:, H // 2 : H, :],
                                    msb[H // 2 : H, :])
                return flat

            # broadcast: gpsimd partition_broadcast halves.  PE-based
            # ones-matmul broadcast saturates the PE (~5.5us/batch; chains
            # then pipeline at ~14us when the write stream needs 7.6us),
            # and the readiness-order list scheduler cannot prioritize the
            # deadline-critical broadcast over a later batch's MLP stages.
            # gpsimd does nothing else, so its queue IS the batch order;
            # its ~6.5us semaphore wake latency overlaps other batches.
            HHW = HW // 2            # 1568 -> 6272B output rows, full rate

            def bcast_half(flat, bc, half):
                lo, hi = (0, HHW) if half == 0 else (HHW, HW)
                nc.gpsimd.partition_broadcast(bc[:, lo:hi], flat[0:1, lo:hi])

            def out_dma_half(b, bc, half):
                lo, hi = (0, HHW) if half == 0 else (HHW, HW)
                for cc in range(NCC):
                    nc.sync.dma_start(out_v[b, cc][:, lo:hi], bc[:, lo:hi])

            bctiles = [bcpool.tile([P, HW], F32, tag="bct", name="bct")
                       for _ in range(BLOC)]

            # ---- emission.  Every batch runs SOLO: batch b's MLP
            # starts as soon as its own two chunks reduce (~4us after its
            # last chunk's completion semaphore), so the long flat->pb
            # handoff (~6us DMA-sem + gpsimd wakeup latency, unavoidable)
            # overlaps the read stream instead of trailing it.  Solo MLPs
            # quadruple the PE's LDWEIGHTS work (~24us total) but the PE
            # has the slack, and the four chains pipeline cleanly through
            # PE -> ACT -> flat-queues -> gpsimd -> sync.
            load_chunk(0, 0)            # DVE reduce
            load_chunk(0, 1)            # ACT; b0 sums ready ~19
            load_chunk(1, 0)            # DVE
            load_chunk(1, 1)            # ACT; b1 sums ready ~27
            nc.sync.dma_start(wpk[:], wpk_d.ap())        # lands ~19
            ysb0 = make_ysb(0, "vector")
            ab0 = mlp_group(0, ysb0)
            flat0 = bilinear_flat(ab0[0])   # flat trigger ~29.5
            load_chunk(2, 0)            # DVE
            ysb1 = make_ysb(1, "vector")
            ab1 = mlp_group(1, ysb1)
            load_chunk(2, 1)            # ACT; b2 sums ready ~34.5
            bcast_half(flat0, bctiles[0], 0)     # gpsimd ~40.5-45.6
            bcast_half(flat0, bctiles[0], 1)
            flat1 = bilinear_flat(ab1[0])
            load_chunk(3, 0)            # DVE
            bcast_half(flat1, bctiles[1], 0)
            load_chunk(3, 1, split=("scalar", "vector"))
            bcast_half(flat1, bctiles[1], 1)
            # output triggers strictly after ALL read triggers (above)
            out_dma_half(0, bctiles[0], 0)       # ~43; reads end ~42
            out_dma_half(0, bctiles[0], 1)
            out_dma_half(1, bctiles[1], 0)
            out_dma_half(1, bctiles[1], 1)
            ysb2 = make_ysb(2, "vector")
            ab2 = mlp_group(2, ysb2)
            flat2 = bilinear_flat(ab2[0])
            bcast_half(flat2, bctiles[2], 0)
            bcast_half(flat2, bctiles[2], 1)
            out_dma_half(2, bctiles[2], 0)   # ready ~53, needed ~58
            out_dma_half(2, bctiles[2], 1)
            ysb3 = make_ysb(3, "vector")     # b3 sums ready ~46
            ab3 = mlp_group(3, ysb3)
            flat3 = bilinear_flat(ab3[0])
            bcast_half(flat3, bctiles[3], 0)
            bcast_half(flat3, bctiles[3], 1)
            out_dma_half(3, bctiles[3], 0)   # ready ~63, needed ~66
            out_dma_half(3, bctiles[3], 1)

    nc.compile()
    return nc


def _prep_in_maps(x, w1, w2, wA, wB):
    x = np.ascontiguousarray(np.asarray(x, dtype=np.float32))
    w1 = np.asarray(w1, dtype=np.float32)
    w2 = np.asarray(w2, dtype=np.float32)
    wA = np.asarray(wA, dtype=np.float32)
    wB = np.asarray(wB, dtype=np.float32)

    bf = ml_dtypes.bfloat16
    w1t = np.ascontiguousarray(w1.T)                       # (C, HIDDEN)
    w2t = np.ascontiguousarray(w2.T)                       # (HIDDEN, C)
    # permute wA rows i*8+r -> r*64+i (8 zero pad cols per r) and wB rows
    # r*56+j -> r*64+j, then transpose: 64-aligned r-chunks make the
    # per-batch (8, 56) bilinear operand extraction a clean strided DMA
    HRP = 2 * C
    wap = np.zeros((RANK, 64, C), np.float32)
    wap[:, :H, :] = wA.reshape(H, RANK, C).transpose(1, 0, 2)
    wat = np.ascontiguousarray(wap.reshape(HRP, C).T)
    wbp = np.zeros((RANK, 64, C), np.float32)
    wbp[:, :W, :] = wB.reshape(RANK, W, C)
    wbt = np.ascontiguousarray(wbp.reshape(HRP, C).T)

    # pack per-partition: [w1t cc-chunks | w2t hh-chunks | wat | wbt]
    def chunked(m, n):          # (n*128, F) -> (128, n*F), chunk-major cols
        f = m.shape[1]
        return m.reshape(n, P, f).transpose(1, 0, 2).reshape(P, n * f)

    wpk = np.concatenate(
        [chunked(w1t, NCC), chunked(w2t, NHH), chunked(wat, NCC),
         chunked(wbt, NCC)], axis=1,
    ).astype(bf)

    xs = x.reshape(NCORES, BLOC, C, HW)
    return [{"x": xs[i], "wpk": wpk} for i in range(NCORES)]


_NC_CACHE = None


def _get_nc():
    global _NC_CACHE
    if _NC_CACHE is None:
        _NC_CACHE = build_bass()
    return _NC_CACHE


def run(inputs: dict, trace: bool = False):
    """Run on 8 NeuronCores. Returns (full_output, BassKernelResults)."""
    in_maps = _prep_in_maps(**inputs)
    nc = _get_nc()
    res = run_bass_kernel_spmd(
        nc, in_maps, core_ids=list(range(NCORES)), trace=trace
    )
    out = np.stack([res.results[i]["out"] for i in range(NCORES)])
    return out.reshape(B, C, H, W).astype(np.float32, copy=False), res


def kernel(x, w1, w2, wA, wB):
    out, _ = run({"x": x, "w1": w1, "w2": w2, "wA": wA, "wB": wB})
    return out



# revision 3
# speedup vs baseline: 1.0591x; 1.0591x over previous
"""AdaptiveBlock Trainium2 kernel, 8-core data-parallel.

Reference (per batch): y = mean(x, HW); y' = gelu(gelu(y@w1.T)@w2.T);
attn = sigmoid((y'@wA.T).reshape(H,R) @ (y'@wB.T).reshape(R,W));
out = attn broadcast over C.

Sharding: B=32 over 8 cores (4 batches/core), weights replicated.
Memory-bound: 13.9 MB read + 12.8 MB written per core; the stream
ramps ~250 -> ~420 GB/s over the first ~25 us (HBM warm-up with all
8 cores phase-aligned), so schedule around a ~350 GB/s read average.

Schedule (trace-driven):
- ALL big HBM traffic rides the single Sync HWDGE queue in program
  order: weights FIRST (1 MB, lands ~15 us -- a trace of the old
  weights-5th order showed them landing ~30 us and shifting every
  MLP -> flat -> broadcast chain ~11 us right, stalling the write
  stream ~8 us), then the 8 x chunks, then all output writes.  The
  queue's FIFO = automatic read priority; writes backfill the
  instant reads drain.
- All 10 read triggers are emitted before any compute so the sync
  queue is fully fed by ~20 us; per-batch compute blocks follow, so
  each engine's in-order queue reads: reduce_k, mlp_k, tanh_k,
  flat_k, reduce_{k+1}, ...  (ACT is on every chain's critical path;
  a reduce_{k+1} emitted before mlp_k's gelus would block them.)
- Spatial sums: cc0 chunk on DVE reduce, cc1 on ACT accum, bf16
  casts for the PE on the engine that produced each sum.  b3's cc1
  chunk is split into two half DMAs: half-chunk completion sems fire
  ~2 us apart vs ~4 us lag for a full chunk, gating the tail chain
  earlier.
- The bilinear A@B is 8 accumulating rank-1 (K=1) matmuls reading
  slices of the batch's A|B row in place (64-aligned host-side weight
  permutation) -- no cross-partition operand extraction.
- sigmoid = Tanh activation + Copy-with-scale/bias affine, both ACT:
  Gelu+Tanh tables coexist in table RAM; Gelu+Sigmoid thrash (1.28 us
  reload per switch on the critical engine).
- The (56,56)->(1,3136) flatten DMA rides the scalar HWDGE queue in
  two 28-packet pieces (trigger cost ~2.3 us total on ACT, but any
  other queue either adds SWDGE sem lag ~4 us or couples the write
  stream head-of-line).
- Broadcast to 128 partitions: gpsimd partition_broadcast in column
  halves (128-partition output is mandatory -- a 64-partition source
  halves the SBUF read fabric and the write stream with it).
- Output DMAs: b0 and b3 column-halved (b0's first half unlocks the
  read->write transition ~2.5 us earlier; b3's halves shorten the
  tail), b1/b2 full-width (12544 B packets run ~4% faster than
  6272 B).  Out triggers are emitted strictly after all read
  triggers (single FIFO: a write trigger ahead of an unissued read
  trigger would head-of-line block it).

Numerics: weights pre-transposed/pre-cast to bf16 host-side, PSUM f32,
activations ~1e-2, tanh values ~1e-4 (bf16/f32-precise near 0): final
rel err ~7e-7 vs the f32 reference.
"""

import numpy as np
import ml_dtypes

import concourse.bass as bass
import concourse.tile as tile
from concourse import bacc, mybir
from concourse.bass_utils import run_bass_kernel_spmd

F32 = mybir.dt.float32
BF16 = mybir.dt.bfloat16

B, C, H, W = 32, 256, 56, 56
HW = H * W                      # 3136
HIDDEN = 512
RANK = 8
HR = H * RANK                   # 448
NCORES = 8
BLOC = B // NCORES              # 4 batches per core
P = 128
NCC = C // P                    # 2 channel chunks
NHH = HIDDEN // P               # 4 hidden chunks
BCHUNK = 512                    # matmul moving free-dim max (PSUM bank)


def build_bass(sim_compat: bool = False) -> bacc.Bacc:
    """sim_compat=True swaps exact Gelu (not implemented in CoreSim) for a
    0.5*x stand-in; with |gelu-input| ~ 0.02 this perturbs the final sigmoid
    output by ~1e-5 relative, so the sim still validates all layout/dataflow.
    Hardware builds always use the exact erf-based Gelu."""
    gelu_f = (
        mybir.ActivationFunctionType.Copy
        if sim_compat
        else mybir.ActivationFunctionType.Gelu
    )
    gelu_s = 0.5 if sim_compat else 1.0
    nc = bacc.Bacc(num_devices=NCORES)

    HRP = 2 * C                                        # 512: r*64+i padded cols
    WPACK = NCC * HIDDEN + NHH * C + 2 * NCC * HRP     # 4096 bf16 columns
    x_d = nc.dram_tensor("x", [BLOC, C, HW], F32, kind="ExternalInput")
    wpk_d = nc.dram_tensor("wpk", [P, WPACK], BF16, kind="ExternalInput")
    out_d = nc.dram_tensor("out", [BLOC, C, HW], F32, kind="ExternalOutput")

    x_v = x_d.ap().rearrange("b (cc p) hw -> b cc p hw", p=P)
    out_v = out_d.ap().rearrange("b (cc p) hw -> b cc p hw", p=P)
    OF_W1 = 0
    OF_W2 = NCC * HIDDEN
    OF_WA = NCC * HIDDEN + NHH * C
    OF_WB = NCC * HIDDEN + NHH * C + NCC * HRP

    with tile.TileContext(nc) as tc:
        with (
            tc.tile_pool(name="xin", bufs=9) as xpool,
            tc.tile_pool(name="persist", bufs=1) as ppool,
            tc.tile_pool(name="small", bufs=2) as spool,
            tc.tile_pool(name="bc", bufs=4) as bcpool,
            tc.tile_pool(name="ps_small", bufs=2, space="PSUM") as ps_small,
            tc.tile_pool(name="ps_ab", bufs=2, space="PSUM") as ps_ab,
        ):
            wpk = ppool.tile([P, WPACK], BF16, tag="wpk", name="wpk")
            w1t = [wpk[:, OF_W1 + cc * HIDDEN : OF_W1 + (cc + 1) * HIDDEN]
                   for cc in range(NCC)]
            w2t = [wpk[:, OF_W2 + hh * C : OF_W2 + (hh + 1) * C]
                   for hh in range(NHH)]
            wat = [wpk[:, OF_WA + cc * HRP : OF_WA + (cc + 1) * HRP]
                   for cc in range(NCC)]
            wbt = [wpk[:, OF_WB + cc * HRP : OF_WB + (cc + 1) * HRP]
                   for cc in range(NCC)]

            # prefetch both activation tables while ACT is idle
            warm = ppool.tile([1, 2], F32, tag="warm", name="warm")
            nc.vector.memset(warm[:], 0.0)
            nc.scalar.activation(warm[:], warm[:],
                                 mybir.ActivationFunctionType.Tanh)
            nc.scalar.activation(warm[:], warm[:], gelu_f)

            ysum = [[ppool.tile([P, 1], F32, tag=f"ysum{b}{cc}",
                                name=f"ysum{b}{cc}")
                     for cc in range(NCC)] for b in range(BLOC)]

            def half_reduce(dst, view, eng):
                if eng == "vector":
                    nc.vector.reduce_sum(dst, view, axis=mybir.AxisListType.X)
                else:
                    nc.scalar.activation(
                        view, view, mybir.ActivationFunctionType.Copy,
                        accum_out=dst,
                    )

            # ---- read triggers: ALL emitted before any compute so the
            # sync FIFO is fully enqueued by ~20 us.  10 DMAs over 8
            # HWDGE sem lanes: lanes recycle onto long-finished
            # transfers, so no trigger stalls.
            nc.sync.dma_start(wpk[:], wpk_d.ap())        # first: lands ~15
            xts = []
            for b in range(BLOC):
                xt = [xpool.tile([P, HW], F32, tag="xt", name=f"xt{b}{cc}")
                      for cc in range(NCC)]
                xts.append(xt)
                nc.sync.dma_start(xt[0][:], x_v[b, 0])
                if b < BLOC - 1:
                    nc.sync.dma_start(xt[1][:], x_v[b, 1])
                else:
                    # tail batch: half DMAs gate the last chain earlier
                    HH = HW // 2
                    nc.sync.dma_start(xt[1][:, 0:HH], x_v[b, 1][:, 0:HH])
                    nc.sync.dma_start(xt[1][:, HH:HW], x_v[b, 1][:, HH:HW])

            def reduce_batch(b):
                """cc0 on DVE, cc1 on ACT; b3's cc1 halves split across
                ACT and DVE so each half reduces behind its own sem."""
                if b < BLOC - 1:
                    half_reduce(ysum[b][0][:], xts[b][0][:], "vector")
                    half_reduce(ysum[b][1][:], xts[b][1][:], "scalar")
                else:
                    HH = HW // 2
                    half_reduce(ysum[b][0][:], xts[b][0][:], "vector")
                    hp = ppool.tile([P, 2], F32, tag=f"hp{b}", name=f"hp{b}")
                    half_reduce(hp[:, 0:1], xts[b][1][:, 0:HH], "scalar")
                    half_reduce(hp[:, 1:2], xts[b][1][:, HH:HW], "vector")
                    nc.vector.tensor_add(ysum[b][1][:], hp[:, 0:1], hp[:, 1:2])

            def make_ysb(b):
                """bf16 casts for PE, each on the engine that made the sum
                (no cross-engine wait inside either queue)."""
                ysb = [ppool.tile([P, 1], BF16, tag=f"ysb{b}{cc}",
                                  name=f"ysb{b}{cc}") for cc in range(NCC)]
                nc.vector.tensor_copy(ysb[0][:], ysum[b][0][:])
                nc.scalar.copy(ysb[1][:], ysum[b][1][:])
                return ysb

            def mlp_batch(b, ysb):
                """MLP + the batch's A|B row."""
                hT = [ppool.tile([P, 1], BF16, tag=f"hT{b}{hh}",
                                 name=f"hT{b}{hh}") for hh in range(NHH)]
                for hh in range(NHH):
                    ph = ps_small.tile([P, 1], F32, tag="ps", name="ps")
                    for cc in range(NCC):
                        nc.tensor.matmul(
                            ph[:], w1t[cc][:, hh * P : (hh + 1) * P], ysb[cc][:],
                            start=(cc == 0), stop=(cc == NCC - 1),
                        )
                    nc.scalar.activation(hT[hh][:], ph[:], gelu_f,
                                         scale=gelu_s / HW)
                ypT = [ppool.tile([P, 1], BF16, tag=f"ypT{b}{cc}",
                                  name=f"ypT{b}{cc}") for cc in range(NCC)]
                for cc in range(NCC):
                    pyp = ps_small.tile([P, 1], F32, tag="ps", name="ps")
                    for hh in range(NHH):
                        nc.tensor.matmul(
                            pyp[:], w2t[hh][:, cc * P : (cc + 1) * P], hT[hh][:],
                            start=(hh == 0), stop=(hh == NHH - 1),
                        )
                    nc.scalar.activation(ypT[cc][:], pyp[:], gelu_f,
                                         scale=gelu_s)
                # A|B projection (M=1): the row sits at partition 0 (PE
                # operand base-partition rule); the bilinear reads slices
                # of this row directly
                pab = ps_ab.tile([1, 2 * BCHUNK], F32, tag="pab", name="pab")
                for half, wt in ((0, wat), (1, wbt)):
                    for cc in range(NCC):
                        nc.tensor.matmul(
                            pab[:, half * BCHUNK : (half + 1) * BCHUNK],
                            ypT[cc][:], wt[cc][:],
                            start=(cc == 0), stop=(cc == NCC - 1),
                        )
                abj = ppool.tile([1, 2 * BCHUNK], BF16, tag=f"ab{b}",
                                 name=f"ab{b}")
                # single-partition copies are free-dim serial: split
                nc.scalar.copy(abj[:, 0:BCHUNK], pab[:, 0:BCHUNK])
                nc.vector.tensor_copy(abj[:, BCHUNK:], pab[:, BCHUNK:])
                return abj

            def bilinear_flat(abj):
                """tanh((A @ B)/2) flattened to (1, HW).  The tanh values
                are ~1e-4; the sigmoid's 0.5x+0.5 rides the broadcast
                PSUM->SBUF copies in f32.

                A @ B is 8 accumulating rank-1 (K=1) matmuls whose operands
                are slices of the batch's ab row itself -- the 64-aligned
                host-side weight permutation puts A[:, r] at cols r*64:+56
                of the first half and B[r, :] at the same cols of the
                second half, so no cross-partition extraction DMA."""
                pm = ps_small.tile([H, W], F32, tag="ps", name="ps")
                for r in range(RANK):
                    nc.tensor.matmul(
                        pm[:],
                        abj[0:1, r * 64 : r * 64 + H],
                        abj[0:1, BCHUNK + r * 64 : BCHUNK + r * 64 + W],
                        start=(r == 0), stop=(r == RANK - 1),
                    )
                msbt = spool.tile([H, W], F32, tag="msbt", name="msbt")
                nc.scalar.activation(msbt[:], pm[:],
                                     mybir.ActivationFunctionType.Tanh,
                                     scale=0.5)
                msb = spool.tile([H, W], F32, tag="msb", name="msb")
                # finishing sigmoid affine on ACT via Copy (no table)
                nc.scalar.activation(msb[:], msbt[:],
                                     mybir.ActivationFunctionType.Copy,
                                     bias=0.5, scale=0.5)
                flat = spool.tile([1, HW], F32, tag="flat", name="flat")
                # two pieces: each bcast half waits only on its own piece
                flat_v = flat[0:1, :].rearrange("o (i j) -> o i j", i=H)
                nc.scalar.dma_start(flat_v[:, 0 : H // 2, :],
                                    msb[0 : H // 2, :])
                nc.scalar.dma_start(flat_v[:, H // 2 : H, :],
                                    msb[H // 2 : H, :])
                return flat

            HHW = HW // 2            # 1568 -> 6272B output rows, full rate

            def bcast_half(flat, bc, half):
                lo, hi = (0, HHW) if half == 0 else (HHW, HW)
                nc.gpsimd.partition_broadcast(bc[:, lo:hi], flat[0:1, lo:hi])

            def out_dma(b, bc, lo, hi):
                for cc in range(NCC):
                    nc.sync.dma_start(out_v[b, cc][:, lo:hi], bc[:, lo:hi])

            bctiles = [bcpool.tile([P, HW], F32, tag="bct", name="bct")
                       for _ in range(BLOC)]

            # ---- per-batch compute blocks.  Each engine's in-order
            # queue sees batch k's full chain before batch k+1's reduce;
            # sums_k complete ~5 us before data_{k+1}'s sem, so the
            # chains pipeline with no head-of-line blocking.
            for b in range(BLOC):
                reduce_batch(b)
                ysb = make_ysb(b)
                abj = mlp_batch(b, ysb)
                flat = bilinear_flat(abj)
                bcast_half(flat, bctiles[b], 0)
                bcast_half(flat, bctiles[b], 1)

            # ---- write triggers, strictly after all read triggers.
            # b0 halved (first half starts the write stream ~2.5 us
            # earlier), b1/b2 full-width, b3 halved for the tail.
            out_dma(0, bctiles[0], 0, HHW)
            out_dma(0, bctiles[0], HHW, HW)
            out_dma(1, bctiles[1], 0, HW)
            out_dma(2, bctiles[2], 0, HW)
            out_dma(3, bctiles[3], 0, HHW)
            out_dma(3, bctiles[3], HHW, HW)

    nc.compile()
    return nc


def _prep_in_maps(x, w1, w2, wA, wB):
    x = np.ascontiguousarray(np.asarray(x, dtype=np.float32))
    w1 = np.asarray(w1, dtype=np.float32)
    w2 = np.asarray(w2, dtype=np.float32)
    wA = np.asarray(wA, dtype=np.float32)
    wB = np.asarray(wB, dtype=np.float32)

    bf = ml_dtypes.bfloat16
    w1t = np.ascontiguousarray(w1.T)                       # (C, HIDDEN)
    w2t = np.ascontiguousarray(w2.T)                       # (HIDDEN, C)
    # permute wA rows i*8+r -> r*64+i (8 zero pad cols per r) and wB rows
    # r*56+j -> r*64+j, then transpose: 64-aligned r-chunks make the
    # per-batch (8, 56) bilinear operand extraction a clean strided DMA
    HRP = 2 * C
    wap = np.zeros((RANK, 64, C), np.float32)
    wap[:, :H, :] = wA.reshape(H, RANK, C).transpose(1, 0, 2)
    wat = np.ascontiguousarray(wap.reshape(HRP, C).T)
    wbp = np.zeros((RANK, 64, C), np.float32)
    wbp[:, :W, :] = wB.reshape(RANK, W, C)
    wbt = np.ascontiguousarray(wbp.reshape(HRP, C).T)

    # pack per-partition: [w1t cc-chunks | w2t hh-chunks | wat | wbt]
    def chunked(m, n):          # (n*128, F) -> (128, n*F), chunk-major cols
        f = m.shape[1]
        return m.reshape(n, P, f).transpose(1, 0, 2).reshape(P, n * f)

    wpk = np.concatenate(
        [chunked(w1t, NCC), chunked(w2t, NHH), chunked(wat, NCC),
         chunked(wbt, NCC)], axis=1,
    ).astype(bf)

    xs = x.reshape(NCORES, BLOC, C, HW)
    return [{"x": xs[i], "wpk": wpk} for i in range(NCORES)]


_NC_CACHE = None


def _get_nc():
    global _NC_CACHE
    if _NC_CACHE is None:
        _NC_CACHE = build_bass()
    return _NC_CACHE


def run(inputs: dict, trace: bool = False):
    """Run on 8 NeuronCores. Returns (full_output, BassKernelResults)."""
    in_maps = _prep_in_maps(**inputs)
    nc = _get_nc()
    res = run_bass_kernel_spmd(
        nc, in_maps, core_ids=list(range(NCORES)), trace=trace
    )
    out = np.stack([res.results[i]["out"] for i in range(NCORES)])
    return out.reshape(B, C, H, W).astype(np.float32, copy=False), res


def kernel(x, w1, w2, wA, wB):
    out, _ = run({"x": x, "w1": w1, "w2": w2, "wA": wA, "wB": wB})
    return out
